# revision 1
# baseline (speedup 1.0000x reference)
"""Causal self-attention kernel for Trainium2, 8 NeuronCores.

Sharding: core c handles batch b = c//2 and head-half c%2 (8 of 16 heads,
512 of 1024 features). Tensor-parallel style: Wq/Wk/Wv split column-wise,
Wp split row-wise; the 2 cores of a batch produce partial outputs that the
host sums (plus the bias const row).

Per-core device program (identical across cores, data differs):
  - projections: qT/kT in [feature, t] layout, v in [s, feature] layout,
    v rows pre-scaled by e^{key_mask_bias} so the exp needs no bias
  - per head-pair (even head on SBUF partitions 0-63, odd head on 64-127):
    S^T[s, t] scores via PE, exp via ACT with fused 1/sqrt(d) scale;
    causal handled by ragged matmuls plus a triangular-mask multiply on
    the diagonal 128x128 block. exp rows live in two 512-wide slots
    (slot = tb % 2), freed by the att@v reads two t-chunks later.
  - att@v FLIPPED: out[t-tile 128, d+1] with lhsT = exp tile (stationary),
    rhs = v (moving, N=65) -- 65-cycle matmuls instead of 512-cycle ones,
    the softmax denominator rides along as column 64 (v col 64 = e^{kb}).
    Normalize with a per-partition reciprocal+scalar-multiply on DVE, then
    PE-transpose (identity matmul) [t,128f] -> [128f,t] into the yT layout
    via a 1-bank PSUM bounce + DVE copy (gpsimd cannot touch PSUM).
  - output projection per t-tile (lagged inside pair 3), staged to fp16
    and stored with one batched DMA per 256 t-rows.
  - emission order doubles as the PE schedule (engines run their queues
    in order), so q/k projection chains and deferred out_proj tiles are
    interleaved as fillers between score chunks to keep the tensor engine
    fed while ACT works through the exps.
"""

import sys

sys.path.insert(0, "/opt/trn_rl_repo")

import numpy as np
import ml_dtypes

import concourse.bass as bass
import concourse.mybir as mybir
import concourse.tile as tile
from concourse import bacc
from concourse.bass_utils import run_bass_kernel_spmd

B, T, C, H = 4, 2048, 1024, 16
D = 64          # head dim
NCORES = 8
NF = 512        # features per core (8 heads)
NH = 8          # heads per core
CT = C // 128   # 8 contraction chunks
NTB = T // 512  # 4 t-blocks
NST = T // 128  # 16 s-tiles
F32 = mybir.dt.float32
F16 = mybir.dt.float16
BF16 = mybir.dt.bfloat16
BF16NP = ml_dtypes.bfloat16

_NC_CACHE = {}
_MARKS = []  # (label, next instruction id) build-phase markers for profiling


def _build():
    nc = bacc.Bacc("TRN2", target_bir_lowering=False, debug=False,
                   num_devices=NCORES)
    xT = nc.dram_tensor("xT", [C, T], BF16, kind="ExternalInput")
    # weights arrive pre-packed in SBUF-tile layout (contiguous DMAs):
    # wqP/wkP[jt*128+p, ci*128+q], wvP[p, ci*512+f], wpP[p, cj*1024+j]
    wqP = nc.dram_tensor("wqP", [NF, C], BF16, kind="ExternalInput")
    wkP = nc.dram_tensor("wkP", [NF, C], BF16, kind="ExternalInput")
    wvP = nc.dram_tensor("wvP", [128, CT * NF], BF16, kind="ExternalInput")
    wpP = nc.dram_tensor("wpP", [128, 4 * C], BF16, kind="ExternalInput")
    cst = nc.dram_tensor("cst", [128, NST + 8], F32, kind="ExternalInput")
    tri = nc.dram_tensor("tri", [128, 256], BF16, kind="ExternalInput")
    idn = nc.dram_tensor("idn", [128, 128], BF16, kind="ExternalInput")
    part = nc.dram_tensor("part", [T, C], F16, kind="ExternalOutput")

    EXP = mybir.ActivationFunctionType.Exp
    SCALE = 1.0 / 8.0  # 1/sqrt(D)

    with tile.TileContext(nc) as tc:
        with (
            tc.tile_pool(name="const", bufs=1) as const,
            tc.tile_pool(name="small", bufs=3) as small,
            tc.tile_pool(name="obp", bufs=2) as obp,
            tc.tile_pool(name="pp_s", bufs=2, space="PSUM") as pp_s,
            tc.tile_pool(name="pp_y", bufs=2, space="PSUM") as pp_y,
            tc.tile_pool(name="pp_a", bufs=2, space="PSUM") as pp_a,
        ):
            # ---- persistent tiles ----
            v_sb = const.tile([128, NST, NH, D + 1], BF16)  # [s_loc, st, h, 65]
            yT_sb = const.tile([128, 4, T], BF16)   # [p, jt, t]
            wp_sb = const.tile([128, 4, C], BF16)   # [p, cj, j]
            tri_sb = const.tile([128, 2, 128], BF16)
            cst_sb = const.tile([128, NST + 8], F32)  # kb[16] | bq[4] | bk[4]
            ekb_sb = const.tile([128, NST], F32)      # e^{kb}
            idn_sb = const.tile([128, 128], BF16)
            ones8 = const.tile([128, NH], F32)

            nc.vector.memset(ones8, 1.0)

            with tc.tile_pool(name="proj", bufs=1) as projp, \
                 tc.tile_pool(name="wjt", bufs=4) as wjtp, \
                 tc.tile_pool(name="qkp", bufs=2) as qkp, \
                 tc.tile_pool(name="wvp", bufs=1) as wvp, \
                 tc.tile_pool(name="expp", bufs=1) as expp:

                def load_wjt(wP, jt, nm, eng=None):
                    # contiguous 2D DMA from the host-packed layout
                    w_jt = wjtp.tile([128, CT, 128], BF16, tag="wjt", name=nm)
                    (eng or nc.sync).dma_start(
                        out=w_jt,
                        in_=wP[jt * 128:(jt + 1) * 128, :].rearrange(
                            "p (ci q) -> p ci q", ci=CT))
                    return w_jt

                # ---- startup: first-needed data first ----
                x_sb = projp.tile([128, CT, T], BF16)

                def load_x(lo, hi, eng):
                    eng.dma_start(
                        out=x_sb[:, :, lo:hi],
                        in_=bass.AP(tensor=xT, offset=lo,
                                    ap=[[T, 128], [128 * T, CT], [1, hi - lo]]))

                load_x(0, 512, nc.sync)          # everything pair-0/tb-0 needs
                wq_jt = load_wjt(wqP, 0, "wq0")
                wk_jt = load_wjt(wkP, 0, "wk0", eng=nc.scalar)
                nc.scalar.dma_start(out=cst_sb, in_=cst.ap())
                nc.scalar.dma_start(
                    out=tri_sb, in_=tri.rearrange("p (u q) -> p u q", u=2))
                nc.scalar.dma_start(out=idn_sb, in_=idn.ap())
                wv_sb = wvp.tile([128, CT, NF], BF16)
                nc.scalar.dma_start(
                    out=wv_sb,
                    in_=wvP.rearrange("p (ci f) -> p ci f", ci=CT))
                load_x(512, 1024, nc.sync)
                load_x(1024, 2048, nc.scalar)
                # e^{kb}: folds the key mask into v rows (softmax identity)
                nc.scalar.activation(ekb_sb, cst_sb[:, 0:NST], EXP)

                # [s_loc, head_parity, sb, t_within_phase]
                exp_sb = expp.tile([128, 2, NST, 1024], BF16)

                def mark(lbl):
                    _MARKS.append((lbl, nc.next_id()))

                def qk_chain(w_jt, dst, bcol, jt, tb):
                    mark(f"chain{jt}_{tb}")
                    ps = pp_y.tile([128, 512], F32, tag="py", name=f"q{jt}{tb}")
                    for ci in range(CT):
                        nc.tensor.matmul(
                            ps,
                            lhsT=w_jt[:, ci, :],
                            rhs=x_sb[:, ci, tb * 512:(tb + 1) * 512],
                            start=(ci == 0), stop=(ci == CT - 1))
                    nc.vector.tensor_scalar_add(
                        dst[:, tb * 512:(tb + 1) * 512], ps,
                        cst_sb[:, bcol + jt:bcol + jt + 1])

                def v_group(st0, st1):
                    mark(f"vgrp{st0}")
                    # v rows scaled by e^{kb[s]}; column 64 holds e^{kb[s]}
                    for st in range(st0, st1):
                        ps = pp_y.tile([128, 512], F32, tag="py", name=f"v{st}")
                        for ci in range(CT):
                            nc.tensor.matmul(
                                ps,
                                lhsT=x_sb[:, ci, st * 128:(st + 1) * 128],
                                rhs=wv_sb[:, ci, :],
                                start=(ci == 0), stop=(ci == CT - 1))
                        nc.vector.tensor_scalar_mul(
                            v_sb[:, st, :, 0:D],
                            ps.rearrange("p (h d) -> p h d", h=NH),
                            ekb_sb[:, st:st + 1])
                        nc.vector.tensor_scalar_mul(
                            v_sb[:, st, :, D:D + 1], ones8,
                            ekb_sb[:, st:st + 1])

                def schunk(hp, qT_t, kT_t, sb, tb):
                    """score chunk [s-tile sb] x [t-block tb], both heads in
                    one 2-bank psum tile; one fused exp ACTIVATE."""
                    mark(f"sch{hp}_{tb}_{sb}")
                    hA, hB = 2 * hp, 2 * hp + 1
                    q_ = {hA: qT_t[0:64, :], hB: qT_t[64:128, :]}
                    k_ = {hA: kT_t[0:64, :], hB: kT_t[64:128, :]}
                    s0, tlo = sb * 128, tb * 512
                    t0 = max(s0, tlo)
                    off = t0 - tlo
                    slot = (tb % 2) * 512
                    ps = pp_s.tile([128, 2, 512], F32, tag="ps",
                                   name=f"s{sb}_{tb}")
                    for h in (hA, hB):
                        nc.tensor.matmul(
                            ps[:, h % 2, off:512],
                            lhsT=k_[h][:, s0:s0 + 128],
                            rhs=q_[h][:, t0:tlo + 512],
                            start=True, stop=True)
                    if s0 < tlo:
                        nc.scalar.activation(
                            exp_sb[:, :, sb, slot + off:slot + 512],
                            ps[:, :, off:512],
                            EXP, scale=SCALE)
                    else:
                        # diagonal 128-block: exp to scratch, then the
                        # triangle-mask multiply WRITES the exp_sb region
                        # (clean RAW chain for the att@v reader)
                        dscr = small.tile([128, 2, 128], BF16, tag="dscr")
                        nc.scalar.activation(
                            dscr, ps[:, :, off:off + 128], EXP, scale=SCALE)
                        nc.vector.tensor_mul(
                            exp_sb[:, :, sb, slot + off:slot + off + 128],
                            dscr, tri_sb)
                        if off + 128 < 512:
                            nc.scalar.activation(
                                exp_sb[:, :, sb,
                                       slot + off + 128:slot + 512],
                                ps[:, :, off + 128:512],
                                EXP, scale=SCALE)

                def attv(hp, tt):
                    """flipped att@v for both heads of pair hp, t-tile tt:
                    out[t 128, 65] accumulated over s-tiles; denominator in
                    column 64; normalize; XBAR-transpose into yT layout."""
                    mark(f"attv{hp}_{tt}")
                    tb = tt // 4
                    slot = (tb % 2) * 512
                    toff = (tt - 4 * tb) * 128
                    y_ps = pp_a.tile([128, 2, D + 1], F32, tag="ya",
                                     name=f"y{hp}_{tt}")
                    for hh in range(2):
                        h = 2 * hp + hh
                        for i in range(tt + 1):
                            nc.tensor.matmul(
                                y_ps[:, hh, :],
                                lhsT=exp_sb[:, hh, i,
                                            slot + toff:slot + toff + 128],
                                rhs=v_sb[:, i, h, :],
                                start=(i == 0), stop=(i == tt))
                    rr = small.tile([128, 2], F32, tag="rr")
                    nc.vector.reciprocal(rr, y_ps[:, :, D:D + 1])
                    y_bf = small.tile([128, 2, D], BF16, tag="ybf")
                    for hh in range(2):
                        nc.vector.tensor_scalar_mul(
                            y_bf[:, hh, :], y_ps[:, hh, 0:D],
                            rr[:, hh:hh + 1])
                    return y_bf

                def ytr(hp, tt, y_bf):
                    """PE-transpose y_bf [t,128f] -> [128f,t] and copy into
                    the yT layout (gpsimd; PSUM tile shares the 'ya' tag)."""
                    mark(f"ytr{hp}_{tt}")
                    tr = pp_a.tile([128, 4 * (D + 1)], BF16, tag="ya",
                                   name=f"tr{hp}_{tt}")
                    nc.tensor.transpose(tr[:, 0:128], y_bf, idn_sb)
                    nc.vector.tensor_copy(
                        out=yT_sb[:, hp, tt * 128:(tt + 1) * 128],
                        in_=tr[:, 0:128])

                ob = {jb: obp.tile([128, 4, 512], F16, tag=f"ob{jb}",
                                   name=f"ob{jb}", bufs=2)
                      for jb in range(2)}

                def out_proj(tt):
                    mark(f"oproj{tt}")
                    # output projection for t-rows [tt*128, (tt+1)*128)
                    for jb in range(2):
                        ps = pp_y.tile([128, 512], F32, tag="py",
                                       name=f"o{tt}{jb}")
                        for cj in range(4):
                            nc.tensor.matmul(
                                ps,
                                lhsT=yT_sb[:, cj, tt * 128:(tt + 1) * 128],
                                rhs=wp_sb[:, cj, jb * 512:(jb + 1) * 512],
                                start=(cj == 0), stop=(cj == 3))
                        nc.any.tensor_copy(out=ob[jb][:, tt % 4, :], in_=ps)
                    if tt % 2 == 1:
                        g, half = tt // 4, (tt % 4) // 2
                        for jb in range(2):
                            nc.scalar.dma_start(
                                out=bass.AP(
                                    tensor=part,
                                    offset=(g * 512 + half * 256) * C
                                    + jb * 512,
                                    ap=[[C, 128], [128 * C, 2], [1, 512]]),
                                in_=ob[jb][:, half * 2:half * 2 + 2, :])
                            if half == 1:
                                ob[jb] = obp.tile(
                                    [128, 4, 512], F16, tag=f"ob{jb}",
                                    name=f"ob{jb}_{g}", bufs=2)

                def pair(hp, ctx, nxt, pre_tbs=(0,)):
                    # ctx = (wq_jt, wk_jt, qT_t, kT_t); the tb0 q/k chains
                    # were already emitted (previous pair's tail or the
                    # prologue).  Own tb1-3 chains and (for hp==0) v_group
                    # units are emitted as PE fillers between schunks so the
                    # tensor engine never starves while ACT works through the
                    # exps (PE executes its queue strictly in program order).
                    wq_jt, wk_jt, qT_t, kT_t = ctx
                    fillers = []
                    for tb in [t for t in range(NTB) if t not in pre_tbs]:
                        fillers.append(("q", tb, lambda tb=tb: qk_chain(
                            wq_jt, qT_t, 16, hp, tb)))
                        fillers.append(("k", tb, lambda tb=tb: qk_chain(
                            wk_jt, kT_t, 20, hp, tb)))

                    def pop_filler():
                        if fillers:
                            fillers.pop(0)[2]()

                    def drain_chains(tb):
                        while fillers and fillers[0][0] in "qk" \
                                and fillers[0][1] <= tb:
                            fillers.pop(0)[2]()

                    pend = []  # t-tiles awaiting transpose: (tt, y_bf)

                    def flush_tr():
                        while pend:
                            tt0, yb = pend.pop(0)
                            ytr(hp, tt0, yb)

                    for tb in range(NTB):
                        drain_chains(tb)
                        for sb in range(4 * tb + 1):  # head schunks
                            schunk(hp, qT_t, kT_t, sb, tb)
                            if sb % 3 == 0 and (tb > 0 or hp > 0):
                                pop_filler()
                        if hp == 0:  # v rows of this tb: needed by its attvs
                            v_group(4 * tb, 4 * tb + 4)
                        for k in range(4):  # tail: schunks + lagged attvs
                            tt = 4 * tb + k
                            if k < 3:
                                schunk(hp, qT_t, kT_t, tt + 1, tb)
                            y_bf = attv(hp, tt)
                            flush_tr()
                            pend.append((tt, y_bf))
                            if hp == 3 and tt >= 2:
                                out_proj(tt - 2)
                        if tb == 2 and nxt is not None:
                            # next pair's first chains fill our tb3 head
                            fillers.append(("n", 9, lambda: qk_chain(
                                nxt[0], nxt[2], 16, hp + 1, 0)))
                            fillers.append(("n", 9, lambda: qk_chain(
                                nxt[1], nxt[3], 20, hp + 1, 0)))
                            if hp >= 1:  # and their tb1 chains too
                                fillers.append(("n", 9, lambda: qk_chain(
                                    nxt[0], nxt[2], 16, hp + 1, 1)))
                                fillers.append(("n", 9, lambda: qk_chain(
                                    nxt[1], nxt[3], 20, hp + 1, 1)))
                    flush_tr()
                    while fillers:
                        fillers.pop(0)[2]()

                def make_ctx(hp):
                    return (load_wjt(wqP, hp, f"wq{hp}"),
                            load_wjt(wkP, hp, f"wk{hp}"),
                            qkp.tile([128, T], BF16, tag="qT", name=f"qT{hp}"),
                            qkp.tile([128, T], BF16, tag="kT", name=f"kT{hp}"))

                ctx = (wq_jt, wk_jt,
                       qkp.tile([128, T], BF16, tag="qT", name="qT0"),
                       qkp.tile([128, T], BF16, tag="kT", name="kT0"))
                qk_chain(ctx[0], ctx[2], 16, 0, 0)
                qk_chain(ctx[1], ctx[3], 20, 0, 0)
                nxt = make_ctx(1)
                pair(0, ctx, nxt, pre_tbs=(0,))
                nc.sync.dma_start(
                    out=wp_sb,
                    in_=wpP.rearrange("p (cj j) -> p cj j", cj=4))
                for hp in range(1, 4):
                    ctx = nxt
                    nxt = make_ctx(hp + 1) if hp < 3 else None
                    pair(hp, ctx, nxt,
                         pre_tbs=(0,) if hp == 1 else (0, 1))
                out_proj(14)
                out_proj(15)

    nc.compile()
    return nc


def _get_nc():
    if "nc" not in _NC_CACHE:
        _NC_CACHE["nc"] = _build()
    return _NC_CACHE["nc"]


def _make_in_maps(x, mask, Wq, bq, Wk, bk, Wv, bv, Wp, bp):
    tri1 = np.triu(np.ones((128, 128), dtype=BF16NP))  # keep s <= t
    tri = np.concatenate([tri1, tri1], axis=1)
    xTs = [np.ascontiguousarray(x[b].T).astype(BF16NP) for b in range(B)]
    def pack_cst(b, F0, F1):
        kb = ((1.0 - mask[b]) * -10000.0).astype(np.float32).reshape(NST, 128).T
        return np.ascontiguousarray(np.concatenate(
            [kb, bq[F0:F1].astype(np.float32).reshape(4, 128).T,
             bk[F0:F1].astype(np.float32).reshape(4, 128).T], axis=1))
    halves = []

    def pack_qk(W, F):
        # [jt*128+p, ci*128+q] <- W[F][jt*128+q, ci*128+p]
        wT = W[F, :].T.astype(BF16NP)            # [C(ci p), NF(jt q)]
        a = wT.reshape(CT, 128, 4, 128)          # [ci, p, jt, q]
        return np.ascontiguousarray(
            a.transpose(2, 1, 0, 3).reshape(NF, C))

    for half in range(2):
        F = slice(half * NF, half * NF + NF)
        wvT = Wv[F, :].T.astype(BF16NP)          # [C, NF]
        wvPk = np.ascontiguousarray(
            wvT.reshape(CT, 128, NF).transpose(1, 0, 2).reshape(128, CT * NF))
        wpT = Wp[:, F].T.astype(BF16NP)          # [NF, C]
        wpPk = np.ascontiguousarray(
            wpT.reshape(4, 128, C).transpose(1, 0, 2).reshape(128, 4 * C))
        halves.append({
            "wqP": pack_qk(Wq, F),
            "wkP": pack_qk(Wk, F),
            "wvP": wvPk,
            "wpP": wpPk,
            "tri": tri,
            "idn": np.eye(128, dtype=BF16NP),
        })
    return [{"xT": xTs[c // 2],
             "cst": pack_cst(c // 2, (c % 2) * NF, (c % 2) * NF + NF),
             **halves[c % 2]}
            for c in range(NCORES)]


def kernel(x, mask, Wq, bq, Wk, bk, Wv, bv, Wp, bp):
    x = np.asarray(x, dtype=np.float32)
    mask = np.asarray(mask, dtype=np.float32)
    Wq, bq = np.asarray(Wq, np.float32), np.asarray(bq, np.float32)
    Wk, bk = np.asarray(Wk, np.float32), np.asarray(bk, np.float32)
    Wv, bv = np.asarray(Wv, np.float32), np.asarray(bv, np.float32)
    Wp, bp = np.asarray(Wp, np.float32), np.asarray(bp, np.float32)

    nc = _get_nc()
    in_maps = _make_in_maps(x, mask, Wq, bq, Wk, bk, Wv, bv, Wp, bp)
    res = run_bass_kernel_spmd(nc, in_maps, list(range(NCORES)))
    const_row = (bv @ Wp.T + bp).astype(np.float32)  # [C]
    out = np.empty((B, T, C), np.float32)
    for b in range(B):
        out[b] = (res.results[2 * b]["part"].astype(np.float32)
                  + res.results[2 * b + 1]["part"].astype(np.float32))
        out[b] += const_row
    return out



# revision 35
# speedup vs baseline: 1.0811x; 1.0811x over previous
"""Causal self-attention kernel for Trainium2, 8 NeuronCores.

Sharding: core c handles batch b = c//2 and head-half c%2 (8 of 16 heads,
512 of 1024 features). Tensor-parallel style: Wq/Wk/Wv split column-wise,
Wp split row-wise; the 2 cores of a batch produce partial outputs that the
host sums (plus the bias const row).

Per-core device program (identical across cores, data differs):
  - projections: qT/kT in [feature, t] layout, v in [s, feature] layout
  - the key-mask bias kb folds into the softmax exp directly as a
    per-partition ACT bias (exp(S/8 + kb_s)), so v needs no prescaling and
    the denominator column of v is a plain 1.0
  - per head-pair (even head on SBUF partitions 0-63, odd head on 64-127):
    S^T[s, t] scores via PE, exp via ACT with fused 1/sqrt(d) scale + kb
    bias; causal handled by ragged matmuls plus an in-place triangular-mask
    multiply (Pool engine) on the diagonal 128x128 block of exp_sb.
  - att@v FLIPPED: out[t-tile 128, d+1] with lhsT = exp tile (stationary),
    rhs = v (moving, N=65) -- 65-cycle matmuls instead of 512-cycle ones,
    the softmax denominator rides along as column 64 (v col 64 = 1.0).
    Normalize with a per-partition reciprocal+scalar-multiply on DVE (ACT
    for the last tiles where ACT is idle), then PE-transpose (identity
    matmul) [t,128f] -> [128f,t] into the yT layout via a 1-bank PSUM
    bounce + DVE copy.
  - output projection per t-tile (interleaved inside pair 3), staged to
    fp16 and stored with one batched DMA per 256 t-rows, split across the
    two HWDGE queues.
  - emission order doubles as the PE schedule (engines run their queues
    in order).  A deficit scheduler tracks queued-PE-minus-queued-ACT time
    and pops filler units (q/k projection chains, per-pair v chains,
    deferred out_proj tiles) whenever the tensor engine would otherwise
    starve while ACT works through the exps.
"""

import sys

sys.path.insert(0, "/opt/trn_rl_repo")

import numpy as np
import ml_dtypes

import concourse.bass as bass
import concourse.mybir as mybir
import concourse.tile as tile
from concourse import bacc
from concourse.bass_utils import run_bass_kernel_spmd

B, T, C, H = 4, 2048, 1024, 16
D = 64          # head dim
NCORES = 8
NF = 512        # features per core (8 heads)
NH = 8          # heads per core
CT = C // 128   # 8 contraction chunks
NTB = T // 512  # 4 t-blocks
NST = T // 128  # 16 s-tiles
F32 = mybir.dt.float32
F16 = mybir.dt.float16
BF16 = mybir.dt.bfloat16
BF16NP = ml_dtypes.bfloat16

_NC_CACHE = {}
_MARKS = []  # (label, next instruction id) build-phase markers for profiling

# deficit-scheduler cost constants (ns, full-speed PE)
PE_COL = 0.4167          # ns per moving column (bf16)
ACT_EL = 0.8333          # ns per element/partition on ACT
ACT_OVH = 217.0          # per-ACT-instruction overhead
CHAIN_NS = 8 * 512 * PE_COL      # one q/k projection chain (per tb)
VUNIT_NS = 8 * 128 * PE_COL      # one per-pair v chain (per s-tile)
OPROJ_NS = 8 * 512 * PE_COL      # one out_proj t-tile
LEAD_MARGIN = 1200.0
TAIL_MARGIN = 2200.0


def _build():
    nc = bacc.Bacc("TRN2", target_bir_lowering=False, debug=False,
                   num_devices=NCORES)
    xT = nc.dram_tensor("xT", [C, T], BF16, kind="ExternalInput")
    # weights arrive pre-packed in SBUF-tile layout (contiguous DMAs):
    # wqP/wkP[jt*128+p, ci*128+q], wvP[p, ci*512+f], wpP[p, cj*1024+j]
    wqP = nc.dram_tensor("wqP", [NF, C], BF16, kind="ExternalInput")
    wkP = nc.dram_tensor("wkP", [NF, C], BF16, kind="ExternalInput")
    wvP = nc.dram_tensor("wvP", [128, CT * NF], BF16, kind="ExternalInput")
    wpP = nc.dram_tensor("wpP", [128, 4 * C], BF16, kind="ExternalInput")
    cst = nc.dram_tensor("cst", [128, NST + 8], F32, kind="ExternalInput")
    tri = nc.dram_tensor("tri", [128, 256], BF16, kind="ExternalInput")
    idn = nc.dram_tensor("idn", [128, 128], BF16, kind="ExternalInput")
    part = nc.dram_tensor("part", [T, C], F16, kind="ExternalOutput")

    EXP = mybir.ActivationFunctionType.Exp
    CPY = mybir.ActivationFunctionType.Copy
    SCALE = 1.0 / 8.0  # 1/sqrt(D)

    with tile.TileContext(nc) as tc:
        with (
            tc.tile_pool(name="const", bufs=1) as const,
            tc.tile_pool(name="small", bufs=4) as small,
            tc.tile_pool(name="obp", bufs=2) as obp,
            tc.tile_pool(name="pp_s", bufs=2, space="PSUM") as pp_s,
            tc.tile_pool(name="pp_y", bufs=2, space="PSUM") as pp_y,
            tc.tile_pool(name="pp_a", bufs=2, space="PSUM") as pp_a,
        ):
            # ---- persistent tiles ----
            v_sb = const.tile([128, NST, NH, D + 1], BF16)  # [s_loc, st, h, 65]
            yT_sb = const.tile([128, 4, T], BF16)   # [p, jt, t]
            wp_sb = const.tile([128, 4, C], BF16)   # [p, cj, j]
            tri_sb = const.tile([128, 2, 128], BF16)
            cst_sb = const.tile([128, NST + 8], F32)  # kb[16] | bq[4] | bk[4]
            idn_sb = const.tile([128, 128], BF16)

            # denominator column of v is constant 1.0
            nc.vector.memset(v_sb[:, :, :, D:D + 1], 1.0)

            with tc.tile_pool(name="proj", bufs=1) as projp, \
                 tc.tile_pool(name="wjt", bufs=4) as wjtp, \
                 tc.tile_pool(name="qkp", bufs=2) as qkp, \
                 tc.tile_pool(name="wvp", bufs=1) as wvp, \
                 tc.tile_pool(name="expp", bufs=1) as expp:

                def load_wjt(wP, jt, nm, eng=None):
                    # contiguous 2D DMA from the host-packed layout
                    w_jt = wjtp.tile([128, CT, 128], BF16, tag="wjt", name=nm)
                    (eng or nc.sync).dma_start(
                        out=w_jt,
                        in_=wP[jt * 128:(jt + 1) * 128, :].rearrange(
                            "p (ci q) -> p ci q", ci=CT))
                    return w_jt

                # ---- startup: first-needed data first, 3 DMA queues ----
                x_sb = projp.tile([128, CT, T], BF16)

                def load_x(lo, hi, eng, c0=0, c1=CT):
                    eng.dma_start(
                        out=x_sb[:, c0:c1, lo:hi],
                        in_=bass.AP(tensor=xT, offset=c0 * 128 * T + lo,
                                    ap=[[T, 128], [128 * T, c1 - c0],
                                        [1, hi - lo]]))

                # PE warmup: burn the slow p-state ramp window on dummy
                # matmuls while the first DMAs are in flight, so the real
                # projection chains run at full clock
                dmy = small.tile([128, 512], BF16, tag="dmy")
                nc.gpsimd.memset(dmy, 0.0)
                for i in range(8):
                    wps = pp_y.tile([128, 512], F32, tag="py",
                                    name=f"warm{i}")
                    nc.tensor.matmul(wps, lhsT=dmy[:, 0:128], rhs=dmy,
                                     start=True, stop=True)

                # the DMA transfer track is globally serialized: order
                # pieces strictly by first need, smallest-first
                wq_jt = load_wjt(wqP, 0, "wq0", eng=nc.sync)
                load_x(0, 512, nc.scalar, 0, 4)   # first q-chain half
                load_x(0, 512, nc.sync, 4, 8)     # second half of tb0
                wk_jt = load_wjt(wkP, 0, "wk0", eng=nc.scalar)
                wv_sb = wvp.tile([128, CT, NF], BF16)
                wv_in = wvP.rearrange("p (ci f) -> p ci f", ci=CT)
                # pair 0's v slice first (needed by its tb0 v chains)
                nc.sync.dma_start(out=wv_sb[:, :, 0:128],
                                  in_=wv_in[:, :, 0:128])
                nc.scalar.dma_start(out=cst_sb, in_=cst.ap())
                # preload the Exp act table off the critical path
                warm = small.tile([128, 1], F32, tag="warm")
                nc.scalar.activation(warm, cst_sb[:, 0:1], EXP)
                nc.scalar.dma_start(
                    out=tri_sb, in_=tri.rearrange("p (u q) -> p u q", u=2))
                nc.scalar.dma_start(out=idn_sb, in_=idn.ap())
                load_x(512, 1024, nc.sync)
                load_x(1024, 1536, nc.scalar)
                load_x(1536, 2048, nc.sync)
                # rest of wv: first needed by pair 1's v chains (~60us in)
                nc.scalar.dma_start(out=wv_sb[:, :, 128:NF],
                                    in_=wv_in[:, :, 128:NF])

                # [s_loc, head_parity, sb, t_within_phase]
                exp_sb = expp.tile([128, 2, NST, 1024], BF16)

                def mark(lbl):
                    _MARKS.append((lbl, nc.next_id()))

                lead = [0.0]  # queued PE minus queued ACT (ns)

                def qk_chain(w_jt, dst, bcol, jt, tb):
                    mark(f"chain{jt}_{tb}")
                    ps = pp_y.tile([128, 512], F32, tag="py", name=f"q{jt}{tb}")
                    for ci in range(CT):
                        nc.tensor.matmul(
                            ps,
                            lhsT=w_jt[:, ci, :],
                            rhs=x_sb[:, ci, tb * 512:(tb + 1) * 512],
                            start=(ci == 0), stop=(ci == CT - 1))
                    nc.vector.tensor_scalar_add(
                        dst[:, tb * 512:(tb + 1) * 512], ps,
                        cst_sb[:, bcol + jt:bcol + jt + 1])
                    lead[0] += CHAIN_NS

                def v_unit(hp, st):
                    # v rows for this pair's 2 heads, one s-tile
                    mark(f"v{hp}_{st}")
                    ps = pp_y.tile([128, 512], F32, tag="py", name=f"v{hp}{st}")
                    for ci in range(CT):
                        nc.tensor.matmul(
                            ps[:, 0:128],
                            lhsT=x_sb[:, ci, st * 128:(st + 1) * 128],
                            rhs=wv_sb[:, ci, hp * 128:(hp + 1) * 128],
                            start=(ci == 0), stop=(ci == CT - 1))
                    nc.vector.tensor_copy(
                        out=v_sb[:, st, 2 * hp:2 * hp + 2, 0:D],
                        in_=ps[:, 0:128].rearrange("p (h d) -> p h d", h=2))
                    lead[0] += VUNIT_NS

                def schunk(hp, qT_t, kT_t, sb, tb):
                    """score chunk [s-tile sb] x [t-block tb], both heads in
                    one 2-bank psum tile; one fused exp ACTIVATE with the
                    key-mask bias; diagonal gets an in-place tri-mask mul."""
                    mark(f"sch{hp}_{tb}_{sb}")
                    hA, hB = 2 * hp, 2 * hp + 1
                    q_ = {hA: qT_t[0:64, :], hB: qT_t[64:128, :]}
                    k_ = {hA: kT_t[0:64, :], hB: kT_t[64:128, :]}
                    s0, tlo = sb * 128, tb * 512
                    t0 = max(s0, tlo)
                    off = t0 - tlo
                    slot = (tb % 2) * 512
                    W = 512 - off
                    ps = pp_s.tile([128, 2, 512], F32, tag="ps",
                                   name=f"s{sb}_{tb}")
                    for h in (hA, hB):
                        nc.tensor.matmul(
                            ps[:, h % 2, off:512],
                            lhsT=k_[h][:, s0:s0 + 128],
                            rhs=q_[h][:, t0:tlo + 512],
                            start=True, stop=True)
                    nc.scalar.activation(
                        exp_sb[:, :, sb, slot + off:slot + 512],
                        ps[:, :, off:512],
                        EXP, scale=SCALE, bias=cst_sb[:, sb:sb + 1])
                    if s0 >= tlo:
                        # diagonal 128-block: in-place triangular mask (Pool)
                        nc.gpsimd.tensor_mul(
                            exp_sb[:, :, sb, slot + off:slot + off + 128],
                            exp_sb[:, :, sb, slot + off:slot + off + 128],
                            tri_sb)
                    lead[0] -= 2 * W * (ACT_EL - PE_COL) + ACT_OVH

                def attv(hp, tt, late=False):
                    """flipped att@v for both heads of pair hp, t-tile tt:
                    out[t 128, 65] accumulated over s-tiles; denominator in
                    column 64; normalize; XBAR-transpose into yT layout."""
                    mark(f"attv{hp}_{tt}")
                    tb = tt // 4
                    slot = (tb % 2) * 512
                    toff = (tt - 4 * tb) * 128
                    y_ps = pp_a.tile([128, 2, D + 1], F32, tag="ya",
                                     name=f"y{hp}_{tt}")
                    # hh-outer: PSUM allows only one pending accumulation
                    # group per bank, so the two heads' groups must not
                    # interleave
                    for hh in range(2):
                        for i in range(tt + 1):
                            nc.tensor.matmul(
                                y_ps[:, hh, :],
                                lhsT=exp_sb[:, hh, i,
                                            slot + toff:slot + toff + 128],
                                rhs=v_sb[:, i, 2 * hp + hh, :],
                                start=(i == 0), stop=(i == tt))
                    rr = small.tile([128, 2], F32, tag="rr")
                    nc.vector.reciprocal(rr, y_ps[:, :, D:D + 1])
                    y_bf = small.tile([128, 2, D], BF16, tag="ybf")
                    for hh in range(2):
                        if late:  # ACT is idle at the pair-3 tail
                            nc.scalar.activation(
                                y_bf[:, hh, :], y_ps[:, hh, 0:D], CPY,
                                scale=rr[:, hh:hh + 1])
                        else:
                            nc.vector.tensor_scalar_mul(
                                y_bf[:, hh, :], y_ps[:, hh, 0:D],
                                rr[:, hh:hh + 1])
                    lead[0] += 2 * (tt + 1) * 65 * PE_COL
                    return y_bf

                def ytr(hp, tt, y_bf):
                    """PE-transpose y_bf [t,128f] -> [128f,t] and copy into
                    the yT layout (PSUM tile shares the 'ya' tag)."""
                    mark(f"ytr{hp}_{tt}")
                    tr = pp_a.tile([128, 4 * (D + 1)], BF16, tag="ya",
                                   name=f"tr{hp}_{tt}")
                    nc.tensor.transpose(tr[:, 0:128], y_bf, idn_sb)
                    nc.vector.tensor_copy(
                        out=yT_sb[:, hp, tt * 128:(tt + 1) * 128],
                        in_=tr[:, 0:128])
                    lead[0] += 128 * PE_COL

                ob = {jb: obp.tile([128, 4, 512], F16, tag=f"ob{jb}",
                                   name=f"ob{jb}", bufs=2)
                      for jb in range(2)}

                def out_proj(tt):
                    mark(f"oproj{tt}")
                    # output projection for t-rows [tt*128, (tt+1)*128)
                    for jb in range(2):
                        ps = pp_y.tile([128, 512], F32, tag="py",
                                       name=f"o{tt}{jb}")
                        for cj in range(4):
                            nc.tensor.matmul(
                                ps,
                                lhsT=yT_sb[:, cj, tt * 128:(tt + 1) * 128],
                                rhs=wp_sb[:, cj, jb * 512:(jb + 1) * 512],
                                start=(cj == 0), stop=(cj == 3))
                        if tt >= 12 and jb == 0:  # ACT idle at the tail
                            nc.scalar.activation(
                                ob[jb][:, tt % 4, :], ps, CPY)
                        else:
                            nc.vector.tensor_copy(
                                out=ob[jb][:, tt % 4, :], in_=ps)
                    if tt >= 12:
                        # tail: flush per t-tile; both on the idle SP queue
                        for jb in range(2):
                            eng = nc.sync
                            eng.dma_start(
                                out=bass.AP(
                                    tensor=part,
                                    offset=tt * 128 * C + jb * 512,
                                    ap=[[C, 128], [1, 512]]),
                                in_=ob[jb][:, tt % 4, :])
                    elif tt % 2 == 1:
                        g, half = tt // 4, (tt % 4) // 2
                        for jb in range(2):
                            eng = nc.sync if jb == 0 else nc.scalar
                            eng.dma_start(
                                out=bass.AP(
                                    tensor=part,
                                    offset=(g * 512 + half * 256) * C
                                    + jb * 512,
                                    ap=[[C, 128], [128 * C, 2], [1, 512]]),
                                in_=ob[jb][:, half * 2:half * 2 + 2, :])
                            if half == 1:
                                ob[jb] = obp.tile(
                                    [128, 4, 512], F16, tag=f"ob{jb}",
                                    name=f"ob{jb}_{g}", bufs=2)
                    lead[0] += OPROJ_NS

                def pair(hp, ctx, nxt, pre_tbs=(0,), donated=(),
                         own_v=True):
                    # ctx = (wq_jt, wk_jt, qT_t, kT_t); the pre_tbs q/k
                    # chains were already emitted (previous pair's fillers
                    # or the prologue).  Remaining own chains, per-pair v
                    # chains, the next pair's first chains, and (hp==3)
                    # out_proj tiles are emitted as PE fillers between
                    # schunks, paced by the lead/deficit accounting so the
                    # tensor engine never starves while ACT works through
                    # the exps (PE executes its queue in program order).
                    wq_jt, wk_jt, qT_t, kT_t = ctx
                    fillers = []  # (kind, key, emit_fn)
                    for tb in [t for t in range(NTB) if t not in pre_tbs]:
                        fillers.append(("q", tb, lambda tb=tb: qk_chain(
                            wq_jt, qT_t, 16, hp, tb)))
                        fillers.append(("k", tb, lambda tb=tb: qk_chain(
                            wk_jt, kT_t, 20, hp, tb)))

                    def pop_filler():
                        if fillers:
                            fillers.pop(0)[2]()
                            return True
                        return False

                    def drain(pred):
                        while True:
                            nxt_f = next((f for f in fillers if pred(f)),
                                         None)
                            if nxt_f is None:
                                return
                            fillers.remove(nxt_f)
                            nxt_f[2]()

                    pend = []    # t-tiles awaiting transpose: (tt, y_bf)
                    pend_o = []  # out_proj units awaiting one more ytr lag

                    def flush_tr(keep=0):
                        while len(pend) > keep:
                            tt0, yb = pend.pop(0)
                            ytr(hp, tt0, yb)
                            if hp == 3:
                                # extra lag: oproj(tt) becomes poppable one
                                # ytr later, so its yT DVE copy has landed
                                fillers.extend(pend_o)
                                pend_o.clear()
                                pend_o.append(("o", tt0,
                                               lambda t=tt0: out_proj(t)))

                    for tb in range(NTB):
                        # own chains for this tb must precede its schunks
                        drain(lambda f, tb=tb: f[0] in "qk" and f[1] <= tb)
                        # this tb's v units: emittable fillers, but must
                        # land before the attv tail below
                        if own_v:
                            vq = [("v", st, lambda st=st: v_unit(hp, st))
                                  for st in range(4 * tb, 4 * tb + 4)]
                            fillers[0:0] = vq
                        if tb == 1 and donated:
                            fillers.extend(donated)
                        for sb in range(4 * tb + 4):  # ALL schunks of tb
                            schunk(hp, qT_t, kT_t, sb, tb)
                            if sb == min(1, 4 * tb):
                                # early v: copies land well before this
                                # tb's attvs read them
                                drain(lambda f, tb=tb: f[0] == "v"
                                      and f[1] <= 4 * tb + 3)
                            while lead[0] < LEAD_MARGIN and pop_filler():
                                pass
                        # v rows of this tb: needed by its attvs
                        drain(lambda f, tb=tb: f[0] == "v"
                              and f[1] <= 4 * tb + 3)
                        for k in range(4):  # attv tail
                            tt = 4 * tb + k
                            # attvs consume just-queued exps: keep a deeper
                            # PE buffer to cover ACT in-flight latency
                            while lead[0] < TAIL_MARGIN and pop_filler():
                                pass
                            y_bf = attv(hp, tt,
                                        late=(hp == 3 and tt >= 14))
                            pend.append((tt, y_bf))
                            flush_tr(keep=3 if tt < 13 else 15 - tt)
                        if tb == 2 and nxt is not None:
                            # next pair's first chains fill our tb3 head
                            fillers.append(("n", 9, lambda: qk_chain(
                                nxt[0], nxt[2], 16, hp + 1, 0)))
                            fillers.append(("n", 9, lambda: qk_chain(
                                nxt[1], nxt[3], 20, hp + 1, 0)))
                            if hp >= 1:  # and their tb1 chains too
                                fillers.append(("n", 9, lambda: qk_chain(
                                    nxt[0], nxt[2], 16, hp + 1, 1)))
                                fillers.append(("n", 9, lambda: qk_chain(
                                    nxt[1], nxt[3], 20, hp + 1, 1)))
                    flush_tr()
                    fillers.extend(pend_o)
                    pend_o.clear()
                    while pop_filler():
                        pass

                def make_ctx(hp):
                    return (load_wjt(wqP, hp, f"wq{hp}"),
                            load_wjt(wkP, hp, f"wk{hp}"),
                            qkp.tile([128, T], BF16, tag="qT", name=f"qT{hp}"),
                            qkp.tile([128, T], BF16, tag="kT", name=f"kT{hp}"))

                ctx = (wq_jt, wk_jt,
                       qkp.tile([128, T], BF16, tag="qT", name="qT0"),
                       qkp.tile([128, T], BF16, tag="kT", name="kT0"))
                qk_chain(ctx[0], ctx[2], 16, 0, 0)
                qk_chain(ctx[1], ctx[3], 20, 0, 0)
                nxt = make_ctx(1)
                pair(0, ctx, nxt, pre_tbs=(0,))
                nc.sync.dma_start(
                    out=wp_sb,
                    in_=wpP.rearrange("p (cj j) -> p cj j", cj=4))
                # pair 3's v chains are donated to pair 2 as extra fillers
                # (pair 3 fills with out_proj instead)
                p3v = [("w", st, lambda st=st: v_unit(3, st))
                       for st in range(NST)]
                for hp in range(1, 4):
                    ctx = nxt
                    nxt = make_ctx(hp + 1) if hp < 3 else None
                    pair(hp, ctx, nxt,
                         pre_tbs=(0,) if hp == 1 else (0, 1),
                         donated=p3v if hp == 2 else (),
                         own_v=(hp != 3))

    nc.compile()
    return nc


def _get_nc():
    if "nc" not in _NC_CACHE:
        _NC_CACHE["nc"] = _build()
    return _NC_CACHE["nc"]


def _make_in_maps(x, mask, Wq, bq, Wk, bk, Wv, bv, Wp, bp):
    tri1 = np.triu(np.ones((128, 128), dtype=BF16NP))  # keep s <= t
    tri = np.concatenate([tri1, tri1], axis=1)
    xTs = [np.ascontiguousarray(x[b].T).astype(BF16NP) for b in range(B)]
    def pack_cst(b, F0, F1):
        kb = ((1.0 - mask[b]) * -10000.0).astype(np.float32).reshape(NST, 128).T
        return np.ascontiguousarray(np.concatenate(
            [kb, bq[F0:F1].astype(np.float32).reshape(4, 128).T,
             bk[F0:F1].astype(np.float32).reshape(4, 128).T], axis=1))
    halves = []

    def pack_qk(W, F):
        # [jt*128+p, ci*128+q] <- W[F][jt*128+q, ci*128+p]
        wT = W[F, :].T.astype(BF16NP)            # [C(ci p), NF(jt q)]
        a = wT.reshape(CT, 128, 4, 128)          # [ci, p, jt, q]
        return np.ascontiguousarray(
            a.transpose(2, 1, 0, 3).reshape(NF, C))

    for half in range(2):
        F = slice(half * NF, half * NF + NF)
        wvT = Wv[F, :].T.astype(BF16NP)          # [C, NF]
        wvPk = np.ascontiguousarray(
            wvT.reshape(CT, 128, NF).transpose(1, 0, 2).reshape(128, CT * NF))
        wpT = Wp[:, F].T.astype(BF16NP)          # [NF, C]
        wpPk = np.ascontiguousarray(
            wpT.reshape(4, 128, C).transpose(1, 0, 2).reshape(128, 4 * C))
        halves.append({
            "wqP": pack_qk(Wq, F),
            "wkP": pack_qk(Wk, F),
            "wvP": wvPk,
            "wpP": wpPk,
            "tri": tri,
            "idn": np.eye(128, dtype=BF16NP),
        })
    return [{"xT": xTs[c // 2],
             "cst": pack_cst(c // 2, (c % 2) * NF, (c % 2) * NF + NF),
             **halves[c % 2]}
            for c in range(NCORES)]


def kernel(x, mask, Wq, bq, Wk, bk, Wv, bv, Wp, bp):
    x = np.asarray(x, dtype=np.float32)
    mask = np.asarray(mask, dtype=np.float32)
    Wq, bq = np.asarray(Wq, np.float32), np.asarray(bq, np.float32)
    Wk, bk = np.asarray(Wk, np.float32), np.asarray(bk, np.float32)
    Wv, bv = np.asarray(Wv, np.float32), np.asarray(bv, np.float32)
    Wp, bp = np.asarray(Wp, np.float32), np.asarray(bp, np.float32)

    nc = _get_nc()
    in_maps = _make_in_maps(x, mask, Wq, bq, Wk, bk, Wv, bv, Wp, bp)
    res = run_bass_kernel_spmd(nc, in_maps, list(range(NCORES)))
    const_row = (bv @ Wp.T + bp).astype(np.float32)  # [C]
    out = np.empty((B, T, C), np.float32)
    for b in range(B):
        out[b] = (res.results[2 * b]["part"].astype(np.float32)
                  + res.results[2 * b + 1]["part"].astype(np.float32))
        out[b] += const_row
    return out


# revision 43
# speedup vs baseline: 1.0943x; 1.0122x over previous
"""Causal self-attention kernel for Trainium2, 8 NeuronCores.

Sharding: core c handles batch b = c//2 and head-half c%2 (8 of 16 heads,
512 of 1024 features). Tensor-parallel style: Wq/Wk/Wv split column-wise,
Wp split row-wise; the 2 cores of a batch produce partial outputs that the
host sums (plus the bias const row).

Per-core device program (identical across cores, data differs):
  - projections: qT/kT in [feature, t] layout, v in [s, feature] layout
  - the key-mask bias kb folds into the softmax exp directly as a
    per-partition ACT bias (exp(S/8 + kb_s)), so v needs no prescaling and
    the denominator column of v is a plain 1.0
  - per head-pair (even head on SBUF partitions 0-63, odd head on 64-127):
    S^T[s, t] scores via PE, exp via ACT with fused 1/sqrt(d) scale + kb
    bias; causal handled by ragged matmuls plus an in-place triangular-mask
    multiply (Pool engine) on the diagonal 128x128 block of exp_sb.
  - att@v FLIPPED: out[t-tile 128, d+1] with lhsT = exp tile (stationary),
    rhs = v (moving, N=65) -- 65-cycle matmuls instead of 512-cycle ones,
    the softmax denominator rides along as column 64 (v col 64 = 1.0).
    Normalize with a per-partition reciprocal+scalar-multiply on DVE (ACT
    for the last tiles where ACT is idle), then PE-transpose (identity
    matmul) [t,128f] -> [128f,t] into the yT layout via a 1-bank PSUM
    bounce + DVE copy.
  - output projection per t-tile (interleaved inside pair 3), staged to
    fp16 and stored with one batched DMA per 256 t-rows, split across the
    two HWDGE queues.
  - emission order doubles as the PE schedule (engines run their queues
    in order).  A deficit scheduler tracks queued-PE-minus-queued-ACT time
    and pops filler units (q/k projection chains, per-pair v chains,
    deferred out_proj tiles) whenever the tensor engine would otherwise
    starve while ACT works through the exps.
"""

import sys

sys.path.insert(0, "/opt/trn_rl_repo")

import numpy as np
import ml_dtypes

import concourse.bass as bass
import concourse.mybir as mybir
import concourse.tile as tile
from concourse import bacc
from concourse.bass_utils import run_bass_kernel_spmd

B, T, C, H = 4, 2048, 1024, 16
D = 64          # head dim
NCORES = 8
NF = 512        # features per core (8 heads)
NH = 8          # heads per core
CT = C // 128   # 8 contraction chunks
NTB = T // 512  # 4 t-blocks
NST = T // 128  # 16 s-tiles
F32 = mybir.dt.float32
F16 = mybir.dt.float16
BF16 = mybir.dt.bfloat16
BF16NP = ml_dtypes.bfloat16

_NC_CACHE = {}
_MARKS = []  # (label, next instruction id) build-phase markers for profiling

# deficit-scheduler cost constants (ns, full-speed PE)
PE_COL = 0.4167          # ns per moving column (bf16)
ACT_EL = 0.8333          # ns per element/partition on ACT
ACT_OVH = 217.0          # per-ACT-instruction overhead
CHAIN_NS = 8 * 512 * PE_COL      # one q/k projection chain (per tb)
VUNIT_NS = 8 * 128 * PE_COL      # one per-pair v chain (per s-tile)
OPROJ_NS = 8 * 512 * PE_COL      # one out_proj t-tile
LEAD_MARGIN = 1200.0
TAIL_MARGIN = 2600.0


def _build():
    nc = bacc.Bacc("TRN2", target_bir_lowering=False, debug=False,
                   num_devices=NCORES)
    xT = nc.dram_tensor("xT", [C, T], BF16, kind="ExternalInput")
    # weights arrive pre-packed in SBUF-tile layout (contiguous DMAs):
    # wqP/wkP[jt*128+p, ci*128+q], wvP[p, ci*512+f], wpP[p, cj*1024+j]
    wqP = nc.dram_tensor("wqP", [NF, C], BF16, kind="ExternalInput")
    wkP = nc.dram_tensor("wkP", [NF, C], BF16, kind="ExternalInput")
    wvP = nc.dram_tensor("wvP", [128, CT * NF], BF16, kind="ExternalInput")
    wpP = nc.dram_tensor("wpP", [128, 4 * C], BF16, kind="ExternalInput")
    cst = nc.dram_tensor("cst", [128, NST + 8], F32, kind="ExternalInput")
    tri = nc.dram_tensor("tri", [128, 256], BF16, kind="ExternalInput")
    idn = nc.dram_tensor("idn", [128, 128], BF16, kind="ExternalInput")
    part = nc.dram_tensor("part", [T, C], F16, kind="ExternalOutput")

    EXP = mybir.ActivationFunctionType.Exp
    CPY = mybir.ActivationFunctionType.Copy
    SCALE = 1.0 / 8.0  # 1/sqrt(D)

    with tile.TileContext(nc) as tc:
        with (
            tc.tile_pool(name="const", bufs=1) as const,
            tc.tile_pool(name="small", bufs=4) as small,
            tc.tile_pool(name="obp", bufs=2) as obp,
            tc.tile_pool(name="pp_s", bufs=2, space="PSUM") as pp_s,
            tc.tile_pool(name="pp_y", bufs=2, space="PSUM") as pp_y,
            tc.tile_pool(name="pp_a", bufs=2, space="PSUM") as pp_a,
        ):
            # ---- persistent tiles ----
            v_sb = const.tile([128, NST, NH, D + 1], BF16)  # [s_loc, st, h, 65]
            yT_sb = const.tile([128, 4, T], BF16)   # [p, jt, t]
            wp_sb = const.tile([128, 4, C], BF16)   # [p, cj, j]
            tri_sb = const.tile([128, 2, 128], BF16)
            cst_sb = const.tile([128, NST + 8], F32)  # kb[16] | bq[4] | bk[4]
            idn_sb = const.tile([128, 128], BF16)

            # denominator column of v is constant 1.0
            nc.vector.memset(v_sb[:, :, :, D:D + 1], 1.0)

            with tc.tile_pool(name="proj", bufs=1) as projp, \
                 tc.tile_pool(name="wjt", bufs=4) as wjtp, \
                 tc.tile_pool(name="qkp", bufs=2) as qkp, \
                 tc.tile_pool(name="wvp", bufs=1) as wvp, \
                 tc.tile_pool(name="expp", bufs=1) as expp:

                def load_wjt(wP, jt, nm, eng=None):
                    # contiguous 2D DMA from the host-packed layout
                    w_jt = wjtp.tile([128, CT, 128], BF16, tag="wjt", name=nm)
                    (eng or nc.sync).dma_start(
                        out=w_jt,
                        in_=wP[jt * 128:(jt + 1) * 128, :].rearrange(
                            "p (ci q) -> p ci q", ci=CT))
                    return w_jt

                # ---- startup: first-needed data first, 3 DMA queues ----
                x_sb = projp.tile([128, CT, T], BF16)

                def load_x(lo, hi, eng, c0=0, c1=CT):
                    eng.dma_start(
                        out=x_sb[:, c0:c1, lo:hi],
                        in_=bass.AP(tensor=xT, offset=c0 * 128 * T + lo,
                                    ap=[[T, 128], [128 * T, c1 - c0],
                                        [1, hi - lo]]))

                # PE warmup: burn the slow p-state ramp window on dummy
                # matmuls while the first DMAs are in flight, so the real
                # projection chains run at full clock
                dmy = small.tile([128, 512], BF16, tag="dmy")
                nc.gpsimd.memset(dmy, 0.0)
                for i in range(8):
                    wps = pp_y.tile([128, 512], F32, tag="py",
                                    name=f"warm{i}")
                    nc.tensor.matmul(wps, lhsT=dmy[:, 0:128], rhs=dmy,
                                     start=True, stop=True)

                # the DMA transfer track is globally serialized: order
                # pieces strictly by first need, smallest-first
                wq_jt = load_wjt(wqP, 0, "wq0", eng=nc.sync)
                load_x(0, 512, nc.scalar, 0, 4)   # first q-chain half
                load_x(0, 512, nc.sync, 4, 8)     # second half of tb0
                wk_jt = load_wjt(wkP, 0, "wk0", eng=nc.scalar)
                wv_sb = wvp.tile([128, CT, NF], BF16)
                wv_in = wvP.rearrange("p (ci f) -> p ci f", ci=CT)
                # pair 0's v slice first (needed by its tb0 v chains)
                nc.sync.dma_start(out=wv_sb[:, :, 0:128],
                                  in_=wv_in[:, :, 0:128])
                nc.scalar.dma_start(out=cst_sb, in_=cst.ap())
                # preload the Exp act table off the critical path
                warm = small.tile([128, 1], F32, tag="warm")
                nc.scalar.activation(warm, cst_sb[:, 0:1], EXP)
                nc.scalar.dma_start(
                    out=tri_sb, in_=tri.rearrange("p (u q) -> p u q", u=2))
                nc.scalar.dma_start(out=idn_sb, in_=idn.ap())
                load_x(512, 1024, nc.sync)
                load_x(1024, 1536, nc.scalar)
                load_x(1536, 2048, nc.sync)
                # rest of wv: first needed by pair 1's v chains (~60us in)
                nc.scalar.dma_start(out=wv_sb[:, :, 128:NF],
                                    in_=wv_in[:, :, 128:NF])

                # [s_loc, head_parity, sb, t_within_phase]
                exp_sb = expp.tile([128, 2, NST, 1024], BF16)

                def mark(lbl):
                    _MARKS.append((lbl, nc.next_id()))

                lead = [0.0]  # queued PE minus queued ACT (ns)

                def qk_chain(w_jt, dst, bcol, jt, tb):
                    mark(f"chain{jt}_{tb}")
                    ps = pp_y.tile([128, 512], F32, tag="py", name=f"q{jt}{tb}")
                    for ci in range(CT):
                        nc.tensor.matmul(
                            ps,
                            lhsT=w_jt[:, ci, :],
                            rhs=x_sb[:, ci, tb * 512:(tb + 1) * 512],
                            start=(ci == 0), stop=(ci == CT - 1))
                    nc.vector.tensor_scalar_add(
                        dst[:, tb * 512:(tb + 1) * 512], ps,
                        cst_sb[:, bcol + jt:bcol + jt + 1])
                    lead[0] += CHAIN_NS

                def v_unit(hp, st):
                    # v rows for this pair's 2 heads, one s-tile
                    mark(f"v{hp}_{st}")
                    ps = pp_y.tile([128, 512], F32, tag="py", name=f"v{hp}{st}")
                    for ci in range(CT):
                        nc.tensor.matmul(
                            ps[:, 0:128],
                            lhsT=x_sb[:, ci, st * 128:(st + 1) * 128],
                            rhs=wv_sb[:, ci, hp * 128:(hp + 1) * 128],
                            start=(ci == 0), stop=(ci == CT - 1))
                    nc.vector.tensor_copy(
                        out=v_sb[:, st, 2 * hp:2 * hp + 2, 0:D],
                        in_=ps[:, 0:128].rearrange("p (h d) -> p h d", h=2))
                    lead[0] += VUNIT_NS

                def schunk(hp, qT_t, kT_t, sb, tb):
                    """score chunk [s-tile sb] x [t-block tb], both heads in
                    one 2-bank psum tile; one fused exp ACTIVATE with the
                    key-mask bias; diagonal gets an in-place tri-mask mul."""
                    mark(f"sch{hp}_{tb}_{sb}")
                    hA, hB = 2 * hp, 2 * hp + 1
                    q_ = {hA: qT_t[0:64, :], hB: qT_t[64:128, :]}
                    k_ = {hA: kT_t[0:64, :], hB: kT_t[64:128, :]}
                    s0, tlo = sb * 128, tb * 512
                    t0 = max(s0, tlo)
                    off = t0 - tlo
                    slot = (tb % 2) * 512
                    W = 512 - off
                    ps = pp_s.tile([128, 2, 512], F32, tag="ps",
                                   name=f"s{sb}_{tb}")
                    for h in (hA, hB):
                        nc.tensor.matmul(
                            ps[:, h % 2, off:512],
                            lhsT=k_[h][:, s0:s0 + 128],
                            rhs=q_[h][:, t0:tlo + 512],
                            start=True, stop=True)
                    nc.scalar.activation(
                        exp_sb[:, :, sb, slot + off:slot + 512],
                        ps[:, :, off:512],
                        EXP, scale=SCALE, bias=cst_sb[:, sb:sb + 1])
                    if s0 >= tlo:
                        # diagonal 128-block: in-place triangular mask (Pool)
                        nc.gpsimd.tensor_mul(
                            exp_sb[:, :, sb, slot + off:slot + off + 128],
                            exp_sb[:, :, sb, slot + off:slot + off + 128],
                            tri_sb)
                    lead[0] -= 2 * W * (ACT_EL - PE_COL) + ACT_OVH

                def attv(hp, tt, late=False):
                    """flipped att@v for both heads of pair hp, t-tile tt:
                    out[t 128, 65] accumulated over s-tiles; denominator in
                    column 64; normalize; XBAR-transpose into yT layout."""
                    mark(f"attv{hp}_{tt}")
                    tb = tt // 4
                    slot = (tb % 2) * 512
                    toff = (tt - 4 * tb) * 128
                    y_ps = pp_a.tile([128, 2, D + 1], F32, tag="ya",
                                     name=f"y{hp}_{tt}")
                    # hh-outer: PSUM allows only one pending accumulation
                    # group per bank, so the two heads' groups must not
                    # interleave
                    for hh in range(2):
                        for i in range(tt + 1):
                            nc.tensor.matmul(
                                y_ps[:, hh, :],
                                lhsT=exp_sb[:, hh, i,
                                            slot + toff:slot + toff + 128],
                                rhs=v_sb[:, i, 2 * hp + hh, :],
                                start=(i == 0), stop=(i == tt))
                    rr = small.tile([128, 2], F32, tag="rr")
                    nc.vector.reciprocal(rr, y_ps[:, :, D:D + 1])
                    y_bf = small.tile([128, 2, D], BF16, tag="ybf")
                    for hh in range(2):
                        if late:  # ACT is idle at the pair-3 tail
                            nc.scalar.activation(
                                y_bf[:, hh, :], y_ps[:, hh, 0:D], CPY,
                                scale=rr[:, hh:hh + 1])
                        else:
                            nc.vector.tensor_scalar_mul(
                                y_bf[:, hh, :], y_ps[:, hh, 0:D],
                                rr[:, hh:hh + 1])
                    lead[0] += 2 * (tt + 1) * 65 * PE_COL
                    return y_bf

                def ytr(hp, tt, y_bf):
                    """PE-transpose y_bf [t,128f] -> [128f,t] and copy into
                    the yT layout (PSUM tile shares the 'ya' tag)."""
                    mark(f"ytr{hp}_{tt}")
                    tr = pp_a.tile([128, 4 * (D + 1)], BF16, tag="ya",
                                   name=f"tr{hp}_{tt}")
                    nc.tensor.transpose(tr[:, 0:128], y_bf, idn_sb)
                    dst = yT_sb[:, hp, tt * 128:(tt + 1) * 128]
                    if hp == 3 and tt >= 12:  # ACT idle at the tail
                        nc.scalar.activation(dst, tr[:, 0:128], CPY)
                    else:
                        nc.vector.tensor_copy(out=dst, in_=tr[:, 0:128])
                    lead[0] += 128 * PE_COL

                ob = {jb: obp.tile([128, 4, 512], F16, tag=f"ob{jb}",
                                   name=f"ob{jb}", bufs=2)
                      for jb in range(2)}

                def out_proj(tt):
                    mark(f"oproj{tt}")
                    # output projection for t-rows [tt*128, (tt+1)*128)
                    for jb in range(2):
                        ps = pp_y.tile([128, 512], F32, tag="py",
                                       name=f"o{tt}{jb}")
                        for cj in range(4):
                            nc.tensor.matmul(
                                ps,
                                lhsT=yT_sb[:, cj, tt * 128:(tt + 1) * 128],
                                rhs=wp_sb[:, cj, jb * 512:(jb + 1) * 512],
                                start=(cj == 0), stop=(cj == 3))
                        if tt >= 12 and jb == 0:  # ACT idle at the tail
                            nc.scalar.activation(
                                ob[jb][:, tt % 4, :], ps, CPY)
                        else:
                            nc.vector.tensor_copy(
                                out=ob[jb][:, tt % 4, :], in_=ps)
                    if tt >= 12:
                        # tail: flush per t-tile; both on the idle SP queue
                        for jb in range(2):
                            eng = nc.sync
                            eng.dma_start(
                                out=bass.AP(
                                    tensor=part,
                                    offset=tt * 128 * C + jb * 512,
                                    ap=[[C, 128], [1, 512]]),
                                in_=ob[jb][:, tt % 4, :])
                    elif tt % 2 == 1:
                        g, half = tt // 4, (tt % 4) // 2
                        for jb in range(2):
                            eng = nc.sync if jb == 0 else nc.scalar
                            eng.dma_start(
                                out=bass.AP(
                                    tensor=part,
                                    offset=(g * 512 + half * 256) * C
                                    + jb * 512,
                                    ap=[[C, 128], [128 * C, 2], [1, 512]]),
                                in_=ob[jb][:, half * 2:half * 2 + 2, :])
                            if half == 1:
                                ob[jb] = obp.tile(
                                    [128, 4, 512], F16, tag=f"ob{jb}",
                                    name=f"ob{jb}_{g}", bufs=2)
                    lead[0] += OPROJ_NS

                def pair(hp, ctx, nxt, pre_tbs=(0,), donated=(),
                         own_v=True):
                    # ctx = (wq_jt, wk_jt, qT_t, kT_t); the pre_tbs q/k
                    # chains were already emitted (previous pair's fillers
                    # or the prologue).  Remaining own chains, per-pair v
                    # chains, the next pair's first chains, and (hp==3)
                    # out_proj tiles are emitted as PE fillers between
                    # schunks, paced by the lead/deficit accounting so the
                    # tensor engine never starves while ACT works through
                    # the exps (PE executes its queue in program order).
                    wq_jt, wk_jt, qT_t, kT_t = ctx
                    fillers = []  # (kind, key, emit_fn)
                    for tb in [t for t in range(NTB) if t not in pre_tbs]:
                        fillers.append(("q", tb, lambda tb=tb: qk_chain(
                            wq_jt, qT_t, 16, hp, tb)))
                        fillers.append(("k", tb, lambda tb=tb: qk_chain(
                            wk_jt, kT_t, 20, hp, tb)))

                    def pop_filler():
                        if fillers:
                            fillers.pop(0)[2]()
                            return True
                        return False

                    def drain(pred):
                        while True:
                            nxt_f = next((f for f in fillers if pred(f)),
                                         None)
                            if nxt_f is None:
                                return
                            fillers.remove(nxt_f)
                            nxt_f[2]()

                    pend = []    # t-tiles awaiting transpose: (tt, y_bf)
                    pend_o = []  # out_proj units awaiting one more ytr lag

                    def flush_tr(keep=0):
                        while len(pend) > keep:
                            tt0, yb = pend.pop(0)
                            ytr(hp, tt0, yb)
                            if hp == 3:
                                # extra lag: oproj(tt) becomes poppable one
                                # ytr later, so its yT DVE copy has landed
                                fillers.extend(pend_o)
                                pend_o.clear()
                                pend_o.append(("o", tt0,
                                               lambda t=tt0: out_proj(t)))

                    for tb in range(NTB):
                        # own chains for this tb must precede its schunks
                        drain(lambda f, tb=tb: f[0] in "qk" and f[1] <= tb)
                        # this tb's v units: emittable fillers, but must
                        # land before the attv tail below
                        if own_v:
                            vq = [("v", st, lambda st=st: v_unit(hp, st))
                                  for st in range(4 * tb, 4 * tb + 4)]
                            fillers[0:0] = vq
                        if tb == 1 and donated:
                            fillers.extend(donated)
                        # diagonal chunks first: their exp -> Pool tri-mul
                        # chain (the longest dependency) completes early,
                        # and the last-emitted chunks are plain exps
                        sb_order = (list(range(4 * tb, 4 * tb + 4))
                                    + list(range(4 * tb)))
                        for si, sb in enumerate(sb_order):
                            schunk(hp, qT_t, kT_t, sb, tb)
                            if si == 1:
                                # early v: copies land well before this
                                # tb's attvs read them
                                drain(lambda f, tb=tb: f[0] == "v"
                                      and f[1] <= 4 * tb + 3)
                            while lead[0] < LEAD_MARGIN and pop_filler():
                                pass
                        # v rows of this tb: needed by its attvs
                        drain(lambda f, tb=tb: f[0] == "v"
                              and f[1] <= 4 * tb + 3)
                        for k in range(4):  # attv tail
                            tt = 4 * tb + k
                            # attvs consume just-queued exps: keep a deeper
                            # PE buffer to cover ACT in-flight latency
                            while lead[0] < TAIL_MARGIN and pop_filler():
                                pass
                            y_bf = attv(hp, tt,
                                        late=(hp == 3 and tt >= 12))
                            pend.append((tt, y_bf))
                            flush_tr(keep=3 if tt < 13 else 15 - tt)
                        if tb == 2 and nxt is not None:
                            # next pair's first chains fill our tb3 head
                            fillers.append(("n", 9, lambda: qk_chain(
                                nxt[0], nxt[2], 16, hp + 1, 0)))
                            fillers.append(("n", 9, lambda: qk_chain(
                                nxt[1], nxt[3], 20, hp + 1, 0)))
                            if hp >= 1:  # and their tb1 chains too
                                fillers.append(("n", 9, lambda: qk_chain(
                                    nxt[0], nxt[2], 16, hp + 1, 1)))
                                fillers.append(("n", 9, lambda: qk_chain(
                                    nxt[1], nxt[3], 20, hp + 1, 1)))
                    flush_tr()
                    fillers.extend(pend_o)
                    pend_o.clear()
                    while pop_filler():
                        pass

                def make_ctx(hp):
                    return (load_wjt(wqP, hp, f"wq{hp}"),
                            load_wjt(wkP, hp, f"wk{hp}"),
                            qkp.tile([128, T], BF16, tag="qT", name=f"qT{hp}"),
                            qkp.tile([128, T], BF16, tag="kT", name=f"kT{hp}"))

                ctx = (wq_jt, wk_jt,
                       qkp.tile([128, T], BF16, tag="qT", name="qT0"),
                       qkp.tile([128, T], BF16, tag="kT", name="kT0"))
                qk_chain(ctx[0], ctx[2], 16, 0, 0)
                qk_chain(ctx[1], ctx[3], 20, 0, 0)
                nxt = make_ctx(1)
                pair(0, ctx, nxt, pre_tbs=(0,))
                nc.sync.dma_start(
                    out=wp_sb,
                    in_=wpP.rearrange("p (cj j) -> p cj j", cj=4))
                # pair 3's v chains are donated to pair 2 as extra fillers
                # (pair 3 fills with out_proj instead)
                p3v = [("w", st, lambda st=st: v_unit(3, st))
                       for st in range(NST)]
                for hp in range(1, 4):
                    ctx = nxt
                    nxt = make_ctx(hp + 1) if hp < 3 else None
                    pair(hp, ctx, nxt,
                         pre_tbs=(0,) if hp == 1 else (0, 1),
                         donated=p3v if hp == 2 else (),
                         own_v=(hp != 3))

    nc.compile()
    return nc


def _get_nc():
    if "nc" not in _NC_CACHE:
        _NC_CACHE["nc"] = _build()
    return _NC_CACHE["nc"]


def _make_in_maps(x, mask, Wq, bq, Wk, bk, Wv, bv, Wp, bp):
    tri1 = np.triu(np.ones((128, 128), dtype=BF16NP))  # keep s <= t
    tri = np.concatenate([tri1, tri1], axis=1)
    xTs = [np.ascontiguousarray(x[b].T).astype(BF16NP) for b in range(B)]
    def pack_cst(b, F0, F1):
        kb = ((1.0 - mask[b]) * -10000.0).astype(np.float32).reshape(NST, 128).T
        return np.ascontiguousarray(np.concatenate(
            [kb, bq[F0:F1].astype(np.float32).reshape(4, 128).T,
             bk[F0:F1].astype(np.float32).reshape(4, 128).T], axis=1))
    halves = []

    def pack_qk(W, F):
        # [jt*128+p, ci*128+q] <- W[F][jt*128+q, ci*128+p]
        wT = W[F, :].T.astype(BF16NP)            # [C(ci p), NF(jt q)]
        a = wT.reshape(CT, 128, 4, 128)          # [ci, p, jt, q]
        return np.ascontiguousarray(
            a.transpose(2, 1, 0, 3).reshape(NF, C))

    for half in range(2):
        F = slice(half * NF, half * NF + NF)
        wvT = Wv[F, :].T.astype(BF16NP)          # [C, NF]
        wvPk = np.ascontiguousarray(
            wvT.reshape(CT, 128, NF).transpose(1, 0, 2).reshape(128, CT * NF))
        wpT = Wp[:, F].T.astype(BF16NP)          # [NF, C]
        wpPk = np.ascontiguousarray(
            wpT.reshape(4, 128, C).transpose(1, 0, 2).reshape(128, 4 * C))
        halves.append({
            "wqP": pack_qk(Wq, F),
            "wkP": pack_qk(Wk, F),
            "wvP": wvPk,
            "wpP": wpPk,
            "tri": tri,
            "idn": np.eye(128, dtype=BF16NP),
        })
    return [{"xT": xTs[c // 2],
             "cst": pack_cst(c // 2, (c % 2) * NF, (c % 2) * NF + NF),
             **halves[c % 2]}
            for c in range(NCORES)]


def kernel(x, mask, Wq, bq, Wk, bk, Wv, bv, Wp, bp):
    x = np.asarray(x, dtype=np.float32)
    mask = np.asarray(mask, dtype=np.float32)
    Wq, bq = np.asarray(Wq, np.float32), np.asarray(bq, np.float32)
    Wk, bk = np.asarray(Wk, np.float32), np.asarray(bk, np.float32)
    Wv, bv = np.asarray(Wv, np.float32), np.asarray(bv, np.float32)
    Wp, bp = np.asarray(Wp, np.float32), np.asarray(bp, np.float32)

    nc = _get_nc()
    in_maps = _make_in_maps(x, mask, Wq, bq, Wk, bk, Wv, bv, Wp, bp)
    res = run_bass_kernel_spmd(nc, in_maps, list(range(NCORES)))
    const_row = (bv @ Wp.T + bp).astype(np.float32)  # [C]
    out = np.empty((B, T, C), np.float32)
    for b in range(B):
        out[b] = (res.results[2 * b]["part"].astype(np.float32)
                  + res.results[2 * b + 1]["part"].astype(np.float32))
        out[b] += const_row
    return out
